# revision 1
# baseline (speedup 1.0000x reference)
"""LoRA QKV with slot routing on 8 TRN2 cores — sorted-token sparse variant.

Host sorts tokens by LoRA slot, so each core's 1024-token shard touches at
most 4 slots (uniform routing over 8 slots; a 1024-token window of the sorted
order can't span more). Per-core LoRA state shrinks to 4 slots x 3 targets
x 16 ranks = 192 local ranks, padded to 256 so every matmul keeps K=M=128
(partial-height matmuls measure ~150ns slower): phase 1 runs 2 matmul groups
instead of 3. B matrices are zero-padded so each target group contracts over
a full 128 partitions. The host un-permutes y afterwards.

Everything else matches the dense kernel: bf16 operands, fp32 PSUM/output,
k-paced sync-ring DMA, W prefetch (bufs=4), stores on the scalar ring.
"""

import numpy as np
import ml_dtypes

import concourse.bass as bass
import concourse.bacc as bacc
import concourse.mybir as mybir
import concourse.tile as tile

HIDDEN = 2048
Q_SIZE = 2048
KV_SIZE = 512
OUT = Q_SIZE + 2 * KV_SIZE  # 3072
MAX_LORAS = 8
RANK = 16
T = 8192
N_CORES = 8
T_CORE = T // N_CORES  # 1024

P = 128
NT = T_CORE // P          # 8 token tiles per core
KC = HIDDEN // P          # 16 k-chunks
OJ = OUT // 512           # 6 output chunks of 512
NSLOT = 4                 # max distinct slots per sorted 1024-token window
LR = NSLOT * RANK         # 64 local ranks per target group
GRP = 2 * P               # 256 = [g0|g1|g2|zero-pad] packed as 2 groups of 128
GR = MAX_LORAS * RANK     # 128 (dense fallback rank count)
F32 = mybir.dt.float32
BF16 = mybir.dt.bfloat16

_NC_CACHE = {}


def build_nc():
    nc = bacc.Bacc("TRN2", target_bir_lowering=False, debug=False, num_devices=N_CORES)

    xT = nc.dram_tensor("xT", [HIDDEN, T_CORE], BF16, kind="ExternalInput").ap()
    wT = nc.dram_tensor("wT", [HIDDEN, OUT], BF16, kind="ExternalInput").ap()
    aT = nc.dram_tensor("aT", [HIDDEN, GRP], BF16, kind="ExternalInput").ap()
    bq = nc.dram_tensor("bq", [P, Q_SIZE], BF16, kind="ExternalInput").ap()
    bk = nc.dram_tensor("bk", [P, KV_SIZE], BF16, kind="ExternalInput").ap()
    bv = nc.dram_tensor("bv", [P, KV_SIZE], BF16, kind="ExternalInput").ap()
    maskT = nc.dram_tensor("maskT", [GRP, T_CORE], BF16, kind="ExternalInput").ap()
    y = nc.dram_tensor("y", [T_CORE, OUT], F32, kind="ExternalOutput").ap()

    with tile.TileContext(nc) as tc:
        with (
            tc.tile_pool(name="xsb", bufs=1) as xpool,
            tc.tile_pool(name="asb", bufs=1) as apool,
            tc.tile_pool(name="bsb", bufs=1) as bpool,
            tc.tile_pool(name="msk", bufs=1) as mpool,
            tc.tile_pool(name="hm", bufs=1) as hmpool,
            tc.tile_pool(name="w", bufs=4) as wpool,
            tc.tile_pool(name="o", bufs=4) as opool,
            tc.tile_pool(name="hps", bufs=1, space="PSUM") as hpsum,
            tc.tile_pool(name="ops", bufs=4, space="PSUM") as opsum,
        ):
            xsb = xpool.tile([P, KC * T_CORE], BF16)   # free idx = k*T_CORE + t
            asb = apool.tile([P, KC * GRP], BF16)      # free idx = k*256 + lr
            aT3 = aT.rearrange("(c p) r -> p c r", p=P)
            xT3 = xT.rearrange("(c p) t -> p c t", p=P)
            asb3 = asb[:].rearrange("p (c r) -> p c r", c=KC)
            xsb3 = xsb[:].rearrange("p (c t) -> p c t", c=KC)
            # pairwise (a, x) delivery paces phase-1 consumption (~1.7us each);
            # first chunk ships alone so the PE starts ~1.5us earlier
            nc.sync.dma_start(asb3[:, 0:1, :], aT3[:, 0:1, :])
            nc.sync.dma_start(xsb3[:, 0:1, :], xT3[:, 0:1, :])
            nc.sync.dma_start(asb3[:, 1:2, :], aT3[:, 1:2, :])
            nc.sync.dma_start(xsb3[:, 1:2, :], xT3[:, 1:2, :])
            for k in range(2, KC, 2):
                nc.sync.dma_start(asb3[:, k:k + 2, :], aT3[:, k:k + 2, :])
                nc.sync.dma_start(xsb3[:, k:k + 2, :], xT3[:, k:k + 2, :])
            msk = mpool.tile([P, 2 * T_CORE], BF16)  # [:, 0:T]=grp1, [:, T:2T]=grp2
            nc.scalar.dma_start(msk[:, 0:T_CORE], maskT[0:P, :])
            nc.scalar.dma_start(msk[:, T_CORE:2 * T_CORE], maskT[P:GRP, :])
            # B rides the sync ring AFTER W j=0 (emitted in the j-loop below):
            # it is not needed until the first B-matmul at ~36us, and keeping
            # it off the scalar ring removes HBM contention during phase 1
            bqsb = bpool.tile([P, Q_SIZE], BF16)
            bksb = bpool.tile([P, KV_SIZE], BF16)
            bvsb = bpool.tile([P, KV_SIZE], BF16)

            # --- phase 1: hT[lr, t] = sum_k A[lr, k] x[t, k]; 2 groups of 128
            # ranks (g0|g1 and g2|pad), token halves sequential (3+ PSUM banks) ---
            hp1 = [hpsum.tile([P, 512], F32, tag=f"hp1{c}", name=f"hp1{c}")
                   for c in range(2)]
            hp2 = [hpsum.tile([P, 512], F32, tag=f"hp2{c}", name=f"hp2{c}")
                   for c in range(2)]
            hm = hmpool.tile([P, 2 * T_CORE], BF16)  # layout mirrors msk
            # k outermost so one (a, x) chunk pair feeds all 4 chains;
            # consecutive matmuls share lhsT across token halves.
            for k in range(KC):
                for grp, hp in ((0, hp1), (1, hp2)):
                    lhsT = asb[:, k * GRP + grp * P: k * GRP + (grp + 1) * P]
                    for hh in range(2):
                        nc.tensor.matmul(
                            hp[hh][:], lhsT=lhsT,
                            rhs=xsb[:, k * T_CORE + hh * 512:
                                    k * T_CORE + (hh + 1) * 512],
                            start=(k == 0), stop=(k == KC - 1))
            for grp, hp in ((0, hp1), (1, hp2)):
                for hh in range(2):
                    nc.vector.tensor_tensor(
                        hm[:, grp * T_CORE + hh * 512: grp * T_CORE + (hh + 1) * 512],
                        hp[hh][:],
                        msk[:, grp * T_CORE + hh * 512: grp * T_CORE + (hh + 1) * 512],
                        op=mybir.AluOpType.mult)

            # --- phase 2: y = x@W^T + hm@B, streamed over 512-wide o-chunks ---
            # hmA rows: g0 ranks 0:64, g1 ranks 64:128 -> bq rows 64:128 are zero
            # and bk rows 0:64 are zero, so each B matmul contracts K=128.
            for j in range(OJ):
                if j < Q_SIZE // 512:
                    grp, bsl = 0, bqsb[:, j * 512:(j + 1) * 512]
                elif j == Q_SIZE // 512:
                    grp, bsl = 0, bksb[:]
                else:
                    grp, bsl = 1, bvsb[:]
                if j == 0:
                    # quarter-tiles for j=0: the first chains start right at
                    # phase-1 end, and quarters land ~1.5us apart instead of
                    # making chain k=8 wait on a whole 1MiB half-tile
                    wq = []
                    for q in range(4):
                        wsb = wpool.tile([P, 4 * 512], BF16, tag="wq",
                                         name="wqsb", bufs=4)
                        nc.sync.dma_start(
                            wsb[:],
                            wT.rearrange("(c p) o -> p c o", p=P)[
                                :, q * 4:(q + 1) * 4, 0:512])
                        wq.append(wsb)

                    def wslice(k, wq=wq):
                        return wq[k // 4][:, (k % 4) * 512:(k % 4 + 1) * 512]
                    # B loads queue behind W j=0 on the sync ring
                    nc.sync.dma_start(bqsb[:], bq[:, :])
                    nc.sync.dma_start(bksb[:], bk[:, :])
                    nc.sync.dma_start(bvsb[:], bv[:, :])
                else:
                    wh = []
                    for half in range(2):
                        wsb = wpool.tile([P, 8 * 512], BF16, tag="w", name="wsb")
                        nc.sync.dma_start(
                            wsb[:],
                            wT.rearrange("(c p) o -> p c o", p=P)[
                                :, half * 8:(half + 1) * 8, j * 512:(j + 1) * 512])
                        wh.append(wsb)

                    def wslice(k, wh=wh):
                        return wh[k // 8][:, (k % 8) * 512:(k % 8 + 1) * 512]
                for i in range(NT):
                    hsl = hm[:, grp * T_CORE + i * P: grp * T_CORE + (i + 1) * P]
                    ops = opsum.tile([P, 512], F32)
                    for k in range(KC):
                        nc.tensor.matmul(
                            ops[:],
                            lhsT=xsb[:, k * T_CORE + i * P: k * T_CORE + (i + 1) * P],
                            rhs=wslice(k),
                            start=(k == 0), stop=False)
                    nc.tensor.matmul(
                        ops[:], lhsT=hsl, rhs=bsl, start=False, stop=True)
                    osb = opool.tile([P, 512], F32)
                    nc.scalar.copy(osb[:], ops[:])
                    nc.scalar.dma_start(
                        y[i * P:(i + 1) * P, j * 512:(j + 1) * 512], osb[:])
    nc.compile()
    return nc


def prep_in_maps(x, weight, lora_A, lora_B_q, lora_B_k, lora_B_v,
                 lora_scaling, token_to_slot):
    bf = ml_dtypes.bfloat16
    x = np.asarray(x, dtype=np.float32)
    lora_A = np.asarray(lora_A, dtype=np.float32)
    Bg = [np.asarray(b, dtype=np.float32) for b in (lora_B_q, lora_B_k, lora_B_v)]
    sc = np.asarray(lora_scaling, dtype=np.float32)
    slot = np.asarray(token_to_slot).astype(np.int64)

    perm = np.argsort(slot, kind="stable")
    slot_s = slot[perm]

    wT = np.ascontiguousarray(weight.T.astype(bf))      # (2048, 3072)

    in_maps = []
    for c in range(N_CORES):
        win = slice(c * T_CORE, (c + 1) * T_CORE)
        toks = perm[win]
        sl = slot_s[win]
        sids = np.unique(sl)
        if len(sids) > NSLOT:
            raise ValueError(f"core {c}: {len(sids)} slots > {NSLOT}")
        sids = np.concatenate([sids, -np.ones(NSLOT - len(sids), np.int64)])

        xTc = np.ascontiguousarray(x[toks].T.astype(bf))  # (2048, 1024)
        # packed rank layout: row g*64 + ls*16 + r for g in {0,1} -> group 1,
        # g=2 at rows 128:192 of group 2, rows 192:256 zero padding.
        a_l = np.zeros((GRP, HIDDEN), np.float32)
        b_l = [np.zeros((P, s), np.float32) for s in (Q_SIZE, KV_SIZE, KV_SIZE)]
        maskTc = np.zeros((GRP, T_CORE), np.float32)  # cast to bf16 on ship-out
        for ls, sid in enumerate(sids):
            if sid < 0:
                continue
            hit = (sl == sid).astype(np.float32)          # (1024,)
            for g in range(3):
                row = g * LR + ls * RANK                  # 0:192 packed
                a_l[row:row + RANK] = lora_A[sid, g]
                maskTc[row:row + RANK] = hit
            b_l[0][ls * RANK:(ls + 1) * RANK] = sc[sid] * Bg[0][sid].T   # g0 -> rows 0:64
            b_l[1][LR + ls * RANK: LR + (ls + 1) * RANK] = sc[sid] * Bg[1][sid].T  # g1 -> 64:128
            b_l[2][ls * RANK:(ls + 1) * RANK] = sc[sid] * Bg[2][sid].T   # g2 -> rows 0:64
        in_maps.append({
            "xT": xTc,
            "wT": wT,
            "aT": np.ascontiguousarray(a_l.T.astype(bf)),
            "bq": np.ascontiguousarray(b_l[0].astype(bf)),
            "bk": np.ascontiguousarray(b_l[1].astype(bf)),
            "bv": np.ascontiguousarray(b_l[2].astype(bf)),
            "maskT": np.ascontiguousarray(maskTc.astype(bf)),
        })
    return in_maps, perm


# --- dense fallback (no token sorting) for pathological slot skew ---



def build_nc_dense():
    """Build the SPMD Bass program (same program on every core)."""
    nc = bacc.Bacc("TRN2", target_bir_lowering=False, debug=False, num_devices=N_CORES)

    xT = nc.dram_tensor("xT", [HIDDEN, T_CORE], BF16, kind="ExternalInput").ap()
    wT = nc.dram_tensor("wT", [HIDDEN, OUT], BF16, kind="ExternalInput").ap()
    aT = nc.dram_tensor("aT", [HIDDEN, 3 * GR], BF16, kind="ExternalInput").ap()
    bq = nc.dram_tensor("bq", [GR, Q_SIZE], BF16, kind="ExternalInput").ap()
    bk = nc.dram_tensor("bk", [GR, KV_SIZE], BF16, kind="ExternalInput").ap()
    bv = nc.dram_tensor("bv", [GR, KV_SIZE], BF16, kind="ExternalInput").ap()
    maskT = nc.dram_tensor("maskT", [GR, T_CORE], F32, kind="ExternalInput").ap()
    y = nc.dram_tensor("y", [T_CORE, OUT], F32, kind="ExternalOutput").ap()

    with tile.TileContext(nc) as tc:
        with (
            tc.tile_pool(name="xsb", bufs=1) as xpool,
            tc.tile_pool(name="asb", bufs=1) as apool,
            tc.tile_pool(name="bsb", bufs=1) as bpool,
            tc.tile_pool(name="msk", bufs=1) as mpool,
            tc.tile_pool(name="hm", bufs=1) as hmpool,
            tc.tile_pool(name="w", bufs=4) as wpool,
            tc.tile_pool(name="o", bufs=4) as opool,
            tc.tile_pool(name="hps", bufs=1, space="PSUM") as hpsum,
            tc.tile_pool(name="ops", bufs=4, space="PSUM") as opsum,
        ):
            xsb = xpool.tile([P, KC * T_CORE], BF16)   # free idx = k*T_CORE + t
            asb = apool.tile([P, KC * 3 * GR], BF16)   # free idx = k*384 + g*128+l*16+r
            # Each dma_start has ~0.6us fixed cost, so batch: A in 2 DMAs,
            # x in k-pair DMAs ordered by consumption (h0 pairs, then h1).
            # mask + B ride the scalar ring, which is idle until stores begin.
            aT3 = aT.rearrange("(c p) r -> p c r", p=P)
            xT3 = xT.rearrange("(c p) t -> p c t", p=P)
            asb3 = asb[:].rearrange("p (c r) -> p c r", c=KC)
            xsb3 = xsb[:].rearrange("p (c t) -> p c t", c=KC)
            # sync ring: pairwise (a, x-h0) paces phase-1 h0; then W follows.
            # scalar ring (idle until stores): x-h1, mask, B.
            for k in range(0, KC, 2):
                nc.sync.dma_start(asb3[:, k:k + 2, :], aT3[:, k:k + 2, :])
                nc.sync.dma_start(
                    xsb3[:, k:k + 2, 0:512], xT3[:, k:k + 2, 0:512])
            for k in range(0, KC, 2):
                nc.sync.dma_start(
                    xsb3[:, k:k + 2, 512:T_CORE], xT3[:, k:k + 2, 512:T_CORE])
            msk = mpool.tile([P, T_CORE], F32)
            nc.scalar.dma_start(msk[:], maskT[:, :])
            bqsb = bpool.tile([P, Q_SIZE], BF16)
            bksb = bpool.tile([P, KV_SIZE], BF16)
            bvsb = bpool.tile([P, KV_SIZE], BF16)
            nc.scalar.dma_start(bqsb[:], bq[:, :])
            nc.scalar.dma_start(bksb[:], bk[:, :])
            nc.scalar.dma_start(bvsb[:], bv[:, :])

            # --- phase 1: hT[g][gr, t] = sum_k A[g][gr, k] x[t, k], k outermost;
            # token halves sequential so only 3 PSUM banks are held ---
            hps = [hpsum.tile([P, 512], F32, tag=f"hps{c}", name=f"hps{c}")
                   for c in range(3)]
            hm = hmpool.tile([P, 3 * T_CORE], BF16)
            for hh in range(2):
                for k in range(KC):
                    for g in range(3):
                        nc.tensor.matmul(
                            hps[g][:],
                            lhsT=asb[:, k * 3 * GR + g * P: k * 3 * GR + (g + 1) * P],
                            rhs=xsb[:, k * T_CORE + hh * 512: k * T_CORE + (hh + 1) * 512],
                            start=(k == 0), stop=(k == KC - 1))
                # mask applied during PSUM drain; hm[g][gr, t] in bf16
                for g in range(3):
                    nc.vector.tensor_tensor(
                        hm[:, g * T_CORE + hh * 512: g * T_CORE + (hh + 1) * 512],
                        hps[g][:], msk[:, hh * 512:(hh + 1) * 512],
                        op=mybir.AluOpType.mult)

            # --- phase 2: y = x@W^T + hm@B, streamed over 512-wide o-chunks ---
            for j in range(OJ):
                if j < Q_SIZE // 512:
                    g, bsl = 0, bqsb[:, j * 512:(j + 1) * 512]
                elif j == Q_SIZE // 512:
                    g, bsl = 1, bksb[:]
                else:
                    g, bsl = 2, bvsb[:]
                wh = []
                for half in range(2):
                    wsb = wpool.tile([P, 8 * 512], BF16, tag="w", name="wsb")
                    # one batched DMA per half-tile: [128p, 8 chunks, 512]
                    nc.sync.dma_start(
                        wsb[:],
                        wT.rearrange("(c p) o -> p c o", p=P)[
                            :, half * 8:(half + 1) * 8, j * 512:(j + 1) * 512])
                    wh.append(wsb)
                for i in range(NT):
                    ops = opsum.tile([P, 512], F32)
                    for k in range(KC):
                        nc.tensor.matmul(
                            ops[:],
                            lhsT=xsb[:, k * T_CORE + i * P: k * T_CORE + (i + 1) * P],
                            rhs=wh[k // 8][:, (k % 8) * 512:(k % 8 + 1) * 512],
                            start=(k == 0), stop=False)
                    nc.tensor.matmul(
                        ops[:],
                        lhsT=hm[:, g * T_CORE + i * P: g * T_CORE + (i + 1) * P],
                        rhs=bsl,
                        start=False, stop=True)
                    osb = opool.tile([P, 512], F32)
                    nc.scalar.copy(osb[:], ops[:])
                    # stores ride the scalar HWDGE ring, separate from W loads
                    nc.scalar.dma_start(
                        y[i * P:(i + 1) * P, j * 512:(j + 1) * 512], osb[:])
    nc.compile()
    return nc


def prep_in_maps_dense(x, weight, lora_A, lora_B_q, lora_B_k, lora_B_v,
                 lora_scaling, token_to_slot):
    bf = ml_dtypes.bfloat16
    x = np.asarray(x, dtype=np.float32)
    lora_scaling = np.asarray(lora_scaling, dtype=np.float32)
    slot = np.asarray(token_to_slot)

    xT = np.ascontiguousarray(np.asarray(x, dtype=np.float32).T.astype(bf))
    wT = np.ascontiguousarray(
        np.asarray(weight, dtype=np.float32).T.astype(bf))          # (2048, 3072)
    # aT col = g*128 + l*16 + r
    aT = np.ascontiguousarray(
        np.asarray(lora_A, dtype=np.float32)
        .transpose(1, 0, 2, 3).reshape(3 * GR, HIDDEN).T.astype(bf))
    # b row = l*16 + r, with scaling folded in
    sc = lora_scaling[:, None, None]
    bq = np.ascontiguousarray(
        (sc * np.asarray(lora_B_q, np.float32)).transpose(0, 2, 1)
        .reshape(GR, Q_SIZE).astype(bf))
    bk = np.ascontiguousarray(
        (sc * np.asarray(lora_B_k, np.float32)).transpose(0, 2, 1)
        .reshape(GR, KV_SIZE).astype(bf))
    bv = np.ascontiguousarray(
        (sc * np.asarray(lora_B_v, np.float32)).transpose(0, 2, 1)
        .reshape(GR, KV_SIZE).astype(bf))
    # one-hot routing mask, repeated over the 16 ranks: maskT[l*16+r, t]
    onehot = (np.arange(MAX_LORAS)[:, None] == slot[None, :]).astype(np.float32)
    maskT = np.repeat(onehot, RANK, axis=0)                         # (128, T)

    in_maps = []
    for c in range(N_CORES):
        sl = slice(c * T_CORE, (c + 1) * T_CORE)
        in_maps.append({
            "xT": np.ascontiguousarray(xT[:, sl]),
            "wT": wT,
            "aT": aT,
            "bq": bq,
            "bk": bk,
            "bv": bv,
            "maskT": np.ascontiguousarray(maskT[:, sl]),
        })
    return in_maps




def kernel(**inputs):
    from concourse.bass_utils import run_bass_kernel_spmd
    try:
        in_maps, perm = prep_in_maps(**inputs)
    except ValueError:
        # >NSLOT distinct slots in some sorted window: use the dense kernel
        if "ncd" not in _NC_CACHE:
            _NC_CACHE["ncd"] = build_nc_dense()
        in_maps = prep_in_maps_dense(**inputs)
        res = run_bass_kernel_spmd(_NC_CACHE["ncd"], in_maps,
                                   core_ids=list(range(N_CORES)))
        return np.concatenate([r["y"] for r in res.results], axis=0)
    if "nc" not in _NC_CACHE:
        _NC_CACHE["nc"] = build_nc()
    res = run_bass_kernel_spmd(_NC_CACHE["nc"], in_maps,
                               core_ids=list(range(N_CORES)))
    y_sorted = np.concatenate([r["y"] for r in res.results], axis=0)
    y = np.empty_like(y_sorted)
    y[perm] = y_sorted
    return y

